# revision 11
# baseline (speedup 1.0000x reference)
"""DeepIRT forward as a Bass/Tile kernel on 8 Trainium2 NeuronCores.

Sharding: pure data parallelism over students (B=4096 -> 8 cores).
Students are globally sorted by qid_len (descending) and dealt to cores so
that every core has an IDENTICAL length profile (dummy students pad the
profile); this lets one SPMD program use a compile-time ragged schedule for
the LSTM (step t only touches the first n_t sorted columns).  Per-length
counts are padded to EVEN so the lo/hi (even/odd student) LSTM column
groups are always the same width (nlo == nhi).

Per-core program layout (P students, P % 16 == 0):
  - students indexed g in [0,P); duo D = g//4 holds 4 students (a = g%4)
  - attention (per "batch" of 4 duos = 16 students):
      qidT comes PRE-TRANSPOSED from the host (no PE transposes), packed
      with kembT (stage-1 lhsT) and kemb2 (stage-2 lhsT) into one DMA
      stage1: scoresT[k,t] (+ mastery preact col) via 4 quadrant matmuls/duo
              + one bias-row matmul adding -1e9 to invalid k rows
      softmax: exp (masked by bias), denominators via ones-matmul,
               reciprocal_approx_fast, then the normalization is folded
               INTO expw via a block-broadcast matmul + one contiguous mul
      stage2: [bvecT | mastvec | avec] via 4 quadrant matmuls/duo; the
              bvecT output is already normalized, so the PSUM->bvec writes
              are plain casts (bvec is student-major: col = pair*T + t,
              contiguous inner dim), split across DVE and GpSimd
  - theta/a DNNs: shared-weight matmuls over all students at once
  - LSTM: 50 steps, students split lo(even g)/hi(odd g) column groups,
          ragged active prefix per step; gate PSUM layout [i|f|o] in one
          3-bank tile + g so sigmoid runs as ONE strided ACT over 3 gates
          (valid because L_b == 0; a per-gate-bias fallback is kept),
          si*tg on GpSimd, rest of cell math on DVE
  - head: b = 4*tanh((h@L_Wo+bo)/2), p = sigmoid(4 * a4 * (theta - b4))

Outputs [1, P] per core are gathered and inverse-permuted on the host.
"""

import sys
import hashlib

import numpy as np
import ml_dtypes

for _p in ("/opt/trn_rl_repo",):
    if _p not in sys.path:
        sys.path.insert(0, _p)

B, T, K, D, H, HL, S, KN = 4096, 50, 32, 64, 256, 128, 100000, 1000
N_CORES = 8
NEG = -1.0e9

_state = {}

BF16 = ml_dtypes.bfloat16


# ---------------------------------------------------------------- host prep

def _host_prep(inputs):
    """Sort/deal/pad students; build per-core input arrays + schedule."""
    lens = np.asarray(inputs["qid_len"]).astype(np.int64)          # [B]
    counts = np.bincount(lens, minlength=T + 1)                    # index 0..50
    m = -(-counts // N_CORES)                                      # ceil
    m[0] = 0
    m[1:] += m[1:] % 2                                             # even counts
    P0 = int(m[1:].sum())
    P = ((P0 + 15) // 16) * 16
    m[1] += P - P0                                                 # 16|P-P0 even

    # per-core identical length profile, descending
    profile = np.repeat(np.arange(T, 0, -1), m[T:0:-1])            # [P]
    assert profile.shape[0] == P

    students = -np.ones((N_CORES, P), np.int64)
    ptr = 0
    for l in range(T, 0, -1):
        idxs = np.where(lens == l)[0]
        for c in range(N_CORES):
            take = idxs[c::N_CORES]
            assert take.shape[0] <= m[l]
            students[c, ptr:ptr + take.shape[0]] = take
        ptr += m[l]
    assert ptr == P

    n_t = np.array([(profile > t).sum() for t in range(T)], np.int64)
    assert np.all(n_t % 2 == 0)
    n2 = [int(x) // 2 for x in n_t]

    ND, NB = P // 4, P // 16

    qidemb = np.asarray(inputs["qidemb"], np.float32)
    stuE = np.asarray(inputs["stuE"], np.float32)
    uididx = np.asarray(inputs["uididx"])
    kcodeidx = np.asarray(inputs["kcodeidx"])
    kcode_len = np.asarray(inputs["kcode_len"]).astype(np.int64)

    lb_np = np.asarray(inputs["L_b"], np.float32)
    bias_zero = bool(np.all(lb_np == 0.0))

    # weights (replicated)
    wts = {
        "wi": np.asarray(inputs["L_Wi"], np.float32).astype(BF16),          # [64,512]
        "wh": np.asarray(inputs["L_Wh"], np.float32).astype(BF16),          # [128,512]
        "tw1": np.asarray(inputs["T_W1"], np.float32).astype(BF16),         # [64,256]
        "aw1": np.asarray(inputs["A_W1"], np.float32).astype(BF16),
        "tw2": np.asarray(inputs["T_W2"], np.float32)[:, 0].reshape(2, 128).T.copy().astype(BF16),
        "aw2": np.asarray(inputs["A_W2"], np.float32)[:, 0].reshape(2, 128).T.copy().astype(BF16),
        "lwo": np.asarray(inputs["L_Wo"], np.float32).reshape(128, 1).astype(BF16),
        "lb": lb_np.reshape(4, 128).T.copy(),
        "tb1": np.asarray(inputs["T_b1"], np.float32).reshape(2, 128).T.copy(),
        "ab1": np.asarray(inputs["A_b1"], np.float32).reshape(2, 128).T.copy(),
        "scal": np.array([[float(np.asarray(inputs["T_b2"]).reshape(-1)[0]),
                           float(np.asarray(inputs["A_b2"]).reshape(-1)[0]),
                           0.5 * float(np.asarray(inputs["L_bo"]).reshape(-1)[0])]],
                         np.float32),
    }
    kne_bf = np.asarray(inputs["knE"], np.float32).astype(BF16)             # [1000,64]
    # constant pattern tiles
    sumpat = np.zeros((128, 4), BF16)
    for a in range(4):
        sumpat[32 * a:32 * (a + 1), a] = 1
    bck = np.zeros((4, 128), np.float32)       # block-broadcast: row a -> rows 32a..
    for a in range(4):
        bck[a, 32 * a:32 * (a + 1)] = 1
    blk4 = np.zeros((4, 204), BF16)
    for j in range(4):
        blk4[j, 51 * j:51 * (j + 1)] = 1
    consts = {"sumpat": sumpat, "bck": bck, "blk4": blk4}

    in_maps = []
    for c in range(N_CORES):
        sel = students[c]
        safe = np.where(sel >= 0, sel, 0)

        q = qidemb[safe]                                           # [P,50,64]
        st = stuE[uididx[safe]]                                    # [P,64]
        qid_plus = np.concatenate([q, st[:, None, :]], axis=1).astype(BF16)
        # host-side transpose: qT[G, d, 102*p8 + 51*par + t]
        qT = qid_plus.reshape(NB, 8, 2, 51, 64).transpose(0, 4, 1, 2, 3) \
            .reshape(NB, 64, 816)

        # pre-gathered knowledge embeddings in both matmul layouts:
        # kemb2[G, 32a+k, 64dd+d] (stage-2 lhsT), kembT[G, d, 128dd+32a+k]
        ke = kne_bf[kcodeidx[safe]]                                # [P,32,64]
        keb = ke.reshape(NB, 4, 4, 32, 64)                         # G,dd,a,k,d
        kemb2 = keb.transpose(0, 2, 3, 1, 4).reshape(NB, 128, 256)
        kembT = keb.transpose(0, 4, 1, 2, 3).reshape(NB, 64, 512)

        # single packed per-G input: [128, 1584]
        #   rows 0:64 cols 0:816    qT
        #   rows 0:64 cols 816:1328 kembT
        #   rows 0:128 cols 1328:1584 kemb2
        qkk = np.zeros((NB, 128, 1584), BF16)
        qkk[:, 0:64, 0:816] = qT
        qkk[:, 0:64, 816:1328] = kembT
        qkk[:, :, 1328:1584] = kemb2

        kl = kcode_len[safe].reshape(ND, 4)                        # [ND,4]
        kk = np.arange(K)
        kmf3 = (kk[None, None, :] < kl[:, :, None])                # [ND,4,32]
        kmf = kmf3.transpose(1, 2, 0).reshape(128, ND).astype(np.float32)
        # brow4[j, G, m] = bias of duo 4G+j at out1 row m (0 valid / -1e9 invalid)
        brow4 = np.where(kmf3, 0.0, NEG).reshape(NB, 4, 128).transpose(1, 0, 2) \
            .reshape(4, NB * 128).copy().astype(BF16)

        im = {"qkk": qkk, "kmf": kmf, "brow": brow4}
        im.update(wts)
        im.update(consts)
        in_maps.append(im)

    meta = {"P": P, "n2": n2, "students": students, "bias_zero": bias_zero}
    return in_maps, meta


# ---------------------------------------------------------------- program

def _build_program(P, n2, bias_zero, phases=("attn", "dnn", "lstm", "head")):
    import os as _os
    if _os.environ.get("KPHASES"):
        phases = tuple(_os.environ["KPHASES"].split(","))
    ATT = int(_os.environ.get("KATT", "9"))
    import concourse.bacc as bacc
    import concourse.bass as bass
    import concourse.tile as tile
    from concourse import mybir
    from concourse.tile import add_dep_helper as add_dep
    from contextlib import ExitStack

    dt = mybir.dt
    AF = mybir.ActivationFunctionType
    ND, NB, NP2 = P // 4, P // 16, P // 2

    nc = bacc.Bacc("TRN2", target_bir_lowering=False, debug=False,
                   enable_asserts=False)

    qkk_d = nc.dram_tensor("qkk", [NB, 128, 1584], dt.bfloat16,
                           kind="ExternalInput")
    kmf_d = nc.dram_tensor("kmf", [128, ND], dt.float32, kind="ExternalInput")
    brow_d = nc.dram_tensor("brow", [4, NB * 128], dt.bfloat16, kind="ExternalInput")
    wi_d = nc.dram_tensor("wi", [64, 512], dt.bfloat16, kind="ExternalInput")
    wh_d = nc.dram_tensor("wh", [128, 512], dt.bfloat16, kind="ExternalInput")
    tw1_d = nc.dram_tensor("tw1", [64, 256], dt.bfloat16, kind="ExternalInput")
    aw1_d = nc.dram_tensor("aw1", [64, 256], dt.bfloat16, kind="ExternalInput")
    tw2_d = nc.dram_tensor("tw2", [128, 2], dt.bfloat16, kind="ExternalInput")
    aw2_d = nc.dram_tensor("aw2", [128, 2], dt.bfloat16, kind="ExternalInput")
    lwo_d = nc.dram_tensor("lwo", [128, 1], dt.bfloat16, kind="ExternalInput")
    lb_d = nc.dram_tensor("lb", [128, 4], dt.float32, kind="ExternalInput")
    tb1_d = nc.dram_tensor("tb1", [128, 2], dt.float32, kind="ExternalInput")
    ab1_d = nc.dram_tensor("ab1", [128, 2], dt.float32, kind="ExternalInput")
    scal_d = nc.dram_tensor("scal", [1, 3], dt.float32, kind="ExternalInput")
    sumpat_d = nc.dram_tensor("sumpat", [128, 4], dt.bfloat16, kind="ExternalInput")
    bck_d = nc.dram_tensor("bck", [4, 128], dt.float32, kind="ExternalInput")
    blk4_d = nc.dram_tensor("blk4", [4, 204], dt.bfloat16, kind="ExternalInput")
    out_d = nc.dram_tensor("out", [1, P], dt.float32, kind="ExternalOutput")

    with tile.TileContext(nc) as tc, ExitStack() as ctx:
        const = ctx.enter_context(tc.tile_pool(name="const", bufs=1))
        state = ctx.enter_context(tc.tile_pool(name="state", bufs=1))

        def load(pool, shape, dty, dram, dma2=False):
            t = pool.tile(shape, dty, tag=f"c_{dram.name}", name=f"c_{dram.name}")
            if dma2:  # duplicate 64-row weight into both partition halves
                nc.sync.dma_start(t[0:64, :], dram.ap())
                nc.sync.dma_start(t[64:128, :], dram.ap())
            else:
                nc.sync.dma_start(t[:], dram.ap())
            return t

        kmf_t = load(const, [128, ND], dt.float32, kmf_d)
        brow_t = load(const, [4, NB * 128], dt.bfloat16, brow_d)
        wi_t = load(const, [128, 512], dt.bfloat16, wi_d, dma2=True)
        wh_t = load(const, [128, 512], dt.bfloat16, wh_d)
        tw1_t = load(const, [128, 256], dt.bfloat16, tw1_d, dma2=True)
        aw1_t = load(const, [128, 256], dt.bfloat16, aw1_d, dma2=True)
        tw2_t = load(const, [128, 2], dt.bfloat16, tw2_d)
        aw2_t = load(const, [128, 2], dt.bfloat16, aw2_d)
        lwo_t = load(const, [128, 1], dt.bfloat16, lwo_d)
        lb_t = load(const, [128, 4], dt.float32, lb_d)
        tb1_t = load(const, [128, 2], dt.float32, tb1_d)
        ab1_t = load(const, [128, 2], dt.float32, ab1_d)
        scal_t = load(const, [1, 3], dt.float32, scal_d)
        sumpat_t = load(const, [128, 4], dt.bfloat16, sumpat_d)
        bck_t = load(const, [4, 128], dt.float32, bck_d)
        blk4_t = load(const, [4, 204], dt.bfloat16, blk4_d)

        # persistent tensors
        # bvec is t-major: col = t*NP2 + pair (pair = 8G+2dd+par), half = g%2
        bvec = state.tile([128, NP2 * T], dt.bfloat16)
        mastav = state.tile([128, P], dt.bfloat16)       # [d(half), 2*slot+c]
        h_t = state.tile([128, P], dt.bfloat16)
        c_t = state.tile([128, P], dt.float32)
        theta_t = state.tile([1, P], dt.float32)
        a4_t = state.tile([1, P], dt.float32)
        b4_t = state.tile([1, P], dt.float32)
        res_t = state.tile([1, P], dt.float32)
        nc.vector.memset(h_t[:], 0.0)
        nc.vector.memset(c_t[:], 0.0)

        # ---------------- attention ----------------
        with ExitStack() as atx:
          if "attn" in phases:
            qin = atx.enter_context(tc.tile_pool(name="qin", bufs=3))
            sbA = atx.enter_context(tc.tile_pool(name="sbA", bufs=3))
            ps_1 = atx.enter_context(tc.tile_pool(name="ps_1", bufs=2, space="PSUM"))
            ps_2e = atx.enter_context(tc.tile_pool(name="ps_2e", bufs=1, space="PSUM"))
            ps_2o = atx.enter_context(tc.tile_pool(name="ps_2o", bufs=1, space="PSUM"))
            ps_d = atx.enter_context(tc.tile_pool(name="ps_d", bufs=2, space="PSUM"))
            ps_b = atx.enter_context(tc.tile_pool(name="ps_b", bufs=2, space="PSUM"))

            bvf2 = bvec[:].rearrange("p (t q two) -> p q two t",
                                     q=NP2 // 2, two=2)

            for G in range(NB):
                # packed per-G input: qT | kembT | kemb2
                qkk = qin.tile([128, 1584], dt.bfloat16, tag="qkk")
                if ATT >= 1:
                    nc.sync.dma_start(qkk[:], qkk_d.ap()[G])
                qT = qkk[0:64, 0:816]
                kT = qkk[0:64, 816:1328]
                kc16 = qkk[:, 1328:1584]
                if ATT < 4:
                    continue

                # stage 1: out1[32a+k, 51*dd+t] = scoresT (+ mastery col 50)
                # first writer: bias matmul filling the whole bank with the
                # -1e9 invalid-k bias (start=True), then 16 quadrant matmuls
                # accumulate the actual scores.
                if ATT < 5:
                    continue
                out1 = ps_1.tile([128, 512], dt.float32, tag="out1")
                bmm = nc.tensor.matmul(
                    out1[:, 0:204], brow_t[:, 128 * G:128 * (G + 1)], blk4_t[:],
                    start=True, stop=False, skip_group_check=True)
                for dd in range(4):
                    for a in range(4):
                        pr = 2 * dd + a // 2
                        rhs = qT[:, 102 * pr + 51 * (a % 2):102 * pr + 51 * (a % 2) + 51]
                        mm = nc.tensor.matmul(
                            out1[32 * a:32 * (a + 1), 51 * dd:51 * (dd + 1)],
                            kT[:, 128 * dd + 32 * a:128 * dd + 32 * (a + 1)],
                            rhs, start=False, stop=(dd == 3 and a == 3),
                            tile_position=(0, 32 * a), skip_group_check=True)
                        add_dep(mm.ins, bmm.ins, reason="bias first-writer")

                # softmax pieces
                if ATT < 6:
                    continue
                o1v = out1[:, 0:204].rearrange("p (d c) -> p d c", d=4)
                expw = sbA.tile([128, 208], dt.bfloat16, tag="expw")
                ewv = expw[:].rearrange("p (d c) -> p d c", d=4)
                nc.scalar.activation(ewv[:, :, 0:50], o1v[:, :, 0:50],
                                     AF.Exp, scale=0.15)
                mast = sbA.tile([128, 4], dt.float32, tag="mast")
                nc.scalar.activation(mast[:], o1v[:, :, 50:51].rearrange("p a o -> p (a o)"),
                                     AF.Sigmoid, scale=0.2)
                nc.gpsimd.tensor_mul(ewv[:, :, 50:51].rearrange("p a o -> p (a o)"),
                                     mast[:], kmf_t[:, 4 * G:4 * G + 4])
                nc.gpsimd.tensor_copy(ewv[:, :, 51:52].rearrange("p a o -> p (a o)"),
                                      kmf_t[:, 4 * G:4 * G + 4])

                # denominators -> reciprocal -> fold into expw
                if ATT < 7:
                    continue
                dps = ps_d.tile([4, 208], dt.float32, tag="dps")
                nc.tensor.matmul(dps[:], sumpat_t[:], expw[:],
                                 start=True, stop=True, skip_group_check=True)
                rden = sbA.tile([4, 208], dt.float32, tag="rden")
                with nc.allow_low_precision(reason="softmax denominators"):
                    nc.vector.reciprocal_approx_fast(rden[:], dps[:])
                # mast (col 50) and kmf (col 51) stay unnormalized
                nc.gpsimd.memset(
                    rden[:].rearrange("p (d c) -> p d c", d=4)[:, :, 50:52], 1.0)
                bc = ps_b.tile([128, 208], dt.float32, tag="bc")
                nc.tensor.matmul(bc[:], bck_t[:], rden[:],
                                 start=True, stop=True, skip_group_check=True)
                nc.vector.tensor_mul(expw[:], expw[:], bc[:])

                # stage 2: [bvecT | mastvec | avec]; expw already normalized,
                # so out2 is the final bvecT. first writer per partition half
                # is start=True (a=0/a=1)
                if ATT < 8:
                    continue
                out2e = ps_2e.tile([128, 512], dt.float32, tag="out2e")
                out2o = ps_2o.tile([128, 512], dt.float32, tag="out2o")
                out2_par = (out2e, out2o)
                firsts = [[None, None], [None, None]]   # [par][hh]
                for dd in range(4):
                    for a in range(4):
                        par = a // 2
                        hh = a % 2
                        o2 = out2_par[par]
                        mm = nc.tensor.matmul(
                            o2[64 * hh:64 * hh + 64, 52 * dd:52 * (dd + 1)],
                            kc16[32 * a:32 * (a + 1), 64 * dd:64 * (dd + 1)],
                            expw[32 * a:32 * (a + 1), 52 * dd:52 * (dd + 1)],
                            start=(firsts[par][hh] is None), stop=True,
                            tile_position=(32 * a, 64 * hh),
                            skip_group_check=True)
                        if firsts[par][hh] is None:
                            firsts[par][hh] = mm
                        else:
                            add_dep(mm.ins, firsts[par][hh].ins,
                                    reason="bank first-writer")

                # write bvec (cast): one [128,4,50] copy per parity; both
                # partition halves of out2 land in the same pair columns
                if ATT < 9:
                    continue
                for par in range(2):
                    src = out2_par[par][:, 0:208].rearrange(
                        "p (d c) -> p d c", d=4)[:, :, 0:50]
                    nc.vector.tensor_copy(bvf2[:, 4 * G:4 * G + 4, par, :], src)

                # mastvec/avec extraction
                mavdst = mastav[:, 16 * G:16 * (G + 1)].rearrange(
                    "p (blk q c) -> p blk q c", blk=4, q=2)
                for par in range(2):
                    src = out2_par[par][:, 0:208].rearrange(
                        "p (d c) -> p d c", d=4)[:, :, 50:52]
                    nc.vector.tensor_copy(mavdst[:, :, par, :], src)

        # ---------------- theta / a DNNs ----------------
        with ExitStack() as dtx:
          if "dnn" in phases:
            sbD = dtx.enter_context(tc.tile_pool(name="sbD", bufs=2))
            ps_h = dtx.enter_context(tc.tile_pool(name="ps_h", bufs=2, space="PSUM"))
            ps_o = dtx.enter_context(tc.tile_pool(name="ps_o", bufs=2, space="PSUM"))

            mav = mastav[:].rearrange("p (s c) -> p s c", s=NP2)
            for net, (w1, b1, w2, sc) in enumerate(
                    [(tw1_t, tb1_t, tw2_t, 0), (aw1_t, ab1_t, aw2_t, 1)]):
                dstv = (theta_t if net == 0 else a4_t)[:].rearrange(
                    "o (q two) -> o q two", two=2)
                for half in range(2):
                    r = slice(64 * half, 64 * half + 64)
                    rhs = mav[r, :, net]                       # [64, NP2]
                    ops = ps_o.tile([1, 512], dt.float32, tag="ops")
                    omm0 = None
                    for b in range(2):
                        hps = ps_h.tile([128, 512], dt.float32, tag="hps")
                        nc.tensor.matmul(hps[:, 0:NP2],
                                         w1[r, 128 * b:128 * (b + 1)], rhs,
                                         start=True, stop=True,
                                         tile_position=(64 * half, 0))
                        t1b = sbD.tile([128, NP2], dt.bfloat16, tag="t1b")
                        nc.scalar.activation(t1b[:], hps[:, 0:NP2], AF.Tanh,
                                             bias=b1[:, b:b + 1])
                        omm = nc.tensor.matmul(ops[:, 0:NP2], w2[:, b:b + 1], t1b[:],
                                               start=(b == 0), stop=(b == 1),
                                               skip_group_check=True)
                        if b == 0:
                            omm0 = omm
                        else:
                            add_dep(omm.ins, omm0.ins,
                                    reason="accum first-writer")
                    nc.scalar.activation(dstv[:, :, half], ops[:, 0:NP2],
                                         AF.Identity, bias=scal_t[:, sc:sc + 1])
            # a4 = tanh(|a_pre| / 2)
            nc.scalar.activation(a4_t[:], a4_t[:], AF.Abs)
            nc.scalar.activation(a4_t[:], a4_t[:], AF.Tanh, scale=0.5)

        # ---------------- LSTM ----------------
        with ExitStack() as ltx:
          if "lstm" in phases:
            ps_g = ltx.enter_context(tc.tile_pool(name="ps_g", bufs=1, space="PSUM"))
            sbL = ltx.enter_context(tc.tile_pool(name="sbL", bufs=2))
            # gate PSUM: per group one [128,1536] (i|f|o) + one [128,512] (g)
            gifo = {}
            gg = {}
            for grp in (0, 1):
                gifo[grp] = ps_g.tile([128, 1536], dt.float32,
                                      tag=f"ifo{grp}", name=f"ifo{grp}")
                gg[grp] = ps_g.tile([128, 512], dt.float32,
                                    tag=f"g{grp}", name=f"g{grp}")
            # (psum_tile, col offset, weight block b) in i,f,o,g order
            for t in range(T):
                n = n2[t]
                if n == 0:
                    continue
                for grp in (0, 1):
                    cb = NP2 * grp
                    wr = 64 * grp
                    regions = [(gifo[grp], 0, 0), (gifo[grp], 512, 1),
                               (gifo[grp], 1024, 3), (gg[grp], 0, 2)]
                    for (gp, off, b) in regions:
                        xrhs = bvec[wr:wr + 64, t * NP2:t * NP2 + n]
                        mmi = nc.tensor.matmul(gp[:, off:off + n],
                                               wi_t[wr:wr + 64, 128 * b:128 * (b + 1)],
                                               xrhs,
                                               start=True, stop=False,
                                               tile_position=(wr, 0),
                                               skip_group_check=True)
                        mmh = nc.tensor.matmul(gp[:, off:off + n],
                                               wh_t[:, 128 * b:128 * (b + 1)],
                                               h_t[:, cb:cb + n],
                                               start=False, stop=True,
                                               skip_group_check=True)
                        add_dep(mmh.ins, mmi.ins, reason="accum first-writer")
                    sfo = sbL.tile([128, 3 * NP2], dt.float32, tag=f"sfo{grp}")
                    vs = sfo[:].rearrange("p (j c) -> p j c", j=3)
                    tg = sbL.tile([128, NP2], dt.float32, tag=f"tg{grp}")
                    giv = gifo[grp][:].rearrange("p (j c) -> p j c", j=3)
                    if bias_zero:
                        nc.scalar.activation(vs[:, :, 0:n], giv[:, :, 0:n],
                                             AF.Sigmoid)
                        nc.scalar.activation(tg[:, 0:n], gg[grp][:, 0:n],
                                             AF.Tanh)
                    else:
                        for j, b in ((0, 0), (1, 1), (2, 3)):
                            nc.scalar.activation(vs[:, j, 0:n], giv[:, j, 0:n],
                                                 AF.Sigmoid, bias=lb_t[:, b:b + 1])
                        nc.scalar.activation(tg[:, 0:n], gg[grp][:, 0:n],
                                             AF.Tanh, bias=lb_t[:, 2:3])
                    t1 = sbL.tile([128, NP2], dt.float32, tag=f"t1{grp}")
                    nc.vector.tensor_mul(t1[:, 0:n], vs[:, 0, 0:n], tg[:, 0:n])
                    nc.vector.tensor_mul(c_t[:, cb:cb + n], c_t[:, cb:cb + n],
                                         vs[:, 1, 0:n])
                    nc.vector.tensor_add(c_t[:, cb:cb + n], c_t[:, cb:cb + n],
                                         t1[:, 0:n])
                    tc2 = sbL.tile([128, NP2], dt.float32, tag=f"tc2{grp}")
                    nc.scalar.activation(tc2[:, 0:n], c_t[:, cb:cb + n], AF.Tanh)
                    nc.vector.tensor_mul(h_t[:, cb:cb + n], vs[:, 2, 0:n],
                                         tc2[:, 0:n])

        # ---------------- head + combine ----------------
        with ExitStack() as htx:
          if "head" in phases:
            ps_r = htx.enter_context(tc.tile_pool(name="ps_r", bufs=2, space="PSUM"))
            sbH = htx.enter_context(tc.tile_pool(name="sbH", bufs=2))
            b4v = b4_t[:].rearrange("o (q two) -> o q two", two=2)
            for half in range(2):
                bps = ps_r.tile([1, 512], dt.float32, tag="bps")
                nc.tensor.matmul(bps[:, 0:NP2], lwo_t[:],
                                 h_t[:, NP2 * half:NP2 * (half + 1)],
                                 start=True, stop=True)
                nc.scalar.activation(b4v[:, :, half], bps[:, 0:NP2], AF.Tanh,
                                     scale=0.5, bias=scal_t[:, 2:3])
            d1 = sbH.tile([1, P], dt.float32, tag="d1")
            # p = sigmoid(a*(t-b)) with a = 4*a4, b = 4*b4
            #   = sigmoid(4 * a4 * (theta - 4*b4))
            nc.vector.scalar_tensor_tensor(d1[:], b4_t[:], -4.0, theta_t[:],
                                           mybir.AluOpType.mult,
                                           mybir.AluOpType.add)
            nc.vector.tensor_mul(d1[:], d1[:], a4_t[:])
            nc.scalar.activation(res_t[:], d1[:], AF.Sigmoid, scale=4.0)
            nc.sync.dma_start(out_d.ap(), res_t[:])

    nc.compile()
    return nc


# ---------------------------------------------------------------- runner

def _fingerprint(inputs):
    h = hashlib.md5()
    for k in sorted(inputs):
        a = np.asarray(inputs[k])
        h.update(k.encode())
        h.update(str(a.shape).encode())
        h.update(str(a.dtype).encode())
        flat = a.reshape(-1)
        stride = max(1, flat.size // 65536)
        h.update(np.ascontiguousarray(flat[::stride]).tobytes())
    return h.digest()


def _install_ntff_hook():
    """Provide antenv.axon_hooks (NTFF profiling over the axon tunnel) when
    the image lacks it: drives libaxon_pjrt.so's profile ABI via ctypes,
    mirroring trn_boot._ntff_profile_via_ctypes."""
    import types
    import ctypes
    import contextlib
    try:
        from antenv.axon_hooks import get_axon_ntff_profile_hook  # noqa: F401
        return True
    except ImportError:
        pass
    so_path = "/opt/axon/libaxon_pjrt.so"
    try:
        lib = ctypes.CDLL(so_path)
    except OSError:
        return False
    if not hasattr(lib, "axon_start_nrt_profile"):
        return False
    lib.axon_start_nrt_profile.argtypes = [ctypes.POINTER(ctypes.c_int64),
                                           ctypes.c_size_t]
    lib.axon_start_nrt_profile.restype = ctypes.c_int64
    lib.axon_stop_nrt_profile.argtypes = [ctypes.c_char_p]
    lib.axon_stop_nrt_profile.restype = ctypes.c_int64

    @contextlib.contextmanager
    def _hook(output_dir, device_ids):
        import jax
        jax.devices()
        if device_ids:
            ids = (ctypes.c_int64 * len(device_ids))(*device_ids)
            rc = lib.axon_start_nrt_profile(ids, len(device_ids))
        else:
            rc = lib.axon_start_nrt_profile(None, 0)
        if rc != 0:
            raise RuntimeError(f"axon_start_nrt_profile rc={rc}")
        try:
            yield
        finally:
            n = lib.axon_stop_nrt_profile(str(output_dir).encode())
            if n < 0:
                raise RuntimeError(f"axon_stop_nrt_profile rc={n}")

    mod = types.ModuleType("antenv.axon_hooks")
    mod.get_axon_ntff_profile_hook = lambda: _hook
    mod.set_axon_ntff_profile_hook = lambda h: None
    import antenv
    sys.modules["antenv.axon_hooks"] = mod
    antenv.axon_hooks = mod
    return True


def profile(trace=True, trace_cores=None):
    """Run the cached program with NTFF tracing; returns BassKernelResults
    (exec_time_ns = on-device NEFF execution time). Call kernel() first."""
    import concourse.bass_utils as bu
    assert "nc" in _state, "call kernel() first to build/caches the program"
    _install_ntff_hook()
    bu.upload_artifacts = lambda d: "local"   # no artifact bucket here
    return bu.run_bass_kernel_spmd(_state["nc"], _state["in_maps"],
                                   core_ids=list(range(N_CORES)), trace=trace,
                                   trace_cores=trace_cores)


def kernel(**inputs):
    from concourse.bass_utils import run_bass_kernel_spmd

    fp = _fingerprint(inputs)
    cached = _state.get("fp")
    if cached != fp:
        in_maps, meta = _host_prep(inputs)
        key = (meta["P"], tuple(meta["n2"]), meta["bias_zero"])
        if _state.get("prog_key") != key:
            _state["nc"] = _build_program(meta["P"], meta["n2"],
                                          meta["bias_zero"])
            _state["prog_key"] = key
        _state["in_maps"] = in_maps
        _state["meta"] = meta
        _state["fp"] = fp

    meta = _state["meta"]
    res = run_bass_kernel_spmd(_state["nc"], _state["in_maps"],
                               core_ids=list(range(N_CORES)))
    out = np.zeros((B, 1), np.float32)
    students = meta["students"]
    for c in range(N_CORES):
        r = res.results[c]["out"].reshape(-1)
        sel = students[c]
        valid = sel >= 0
        out[sel[valid], 0] = r[:len(sel)][valid]
    return out


# revision 19
# speedup vs baseline: 1.2624x; 1.2624x over previous
"""DeepIRT forward as a Bass/Tile kernel on 8 Trainium2 NeuronCores.

Sharding: pure data parallelism over students (B=4096 -> 8 cores).
Students are globally sorted by qid_len (descending) and dealt to cores so
that every core has an IDENTICAL length profile (dummy students pad the
profile); this lets one SPMD program use a compile-time ragged schedule for
the LSTM (step t only touches the first n_t sorted columns).  Per-length
counts are padded to EVEN so the lo/hi (even/odd student) LSTM column
groups are always the same width (nlo == nhi).

Per-core program layout (P students, P % 16 == 0):
  - students indexed g in [0,P); duo D = g//4 holds 4 students (a = g%4)
  - attention (per "batch" of 4 duos = 16 students):
      qidT comes PRE-TRANSPOSED from the host (no PE transposes), packed
      with kembT (stage-1 lhsT) and kemb2 (stage-2 lhsT) into one DMA
      stage1: scoresT[k,t] (+ mastery preact col) via 4 quadrant matmuls/duo
              + one bias-row matmul adding -1e9 to invalid k rows
      softmax: exp (masked by bias), denominators via ones-matmul,
               reciprocal_approx_fast, then the normalization is folded
               INTO expw via a block-broadcast matmul + one contiguous mul
      stage2: [bvecT | mastvec | avec] via 4 quadrant matmuls/duo; the
              bvecT output is already normalized, so the PSUM->bvec writes
              are plain casts (bvec is student-major: col = pair*T + t,
              contiguous inner dim), split across DVE and GpSimd
  - theta/a DNNs: shared-weight matmuls over all students at once
  - LSTM: 50 steps, students split lo(even g)/hi(odd g) column groups,
          ragged active prefix per step; gate PSUM layout [i|f|o] in one
          3-bank tile + g so sigmoid runs as ONE strided ACT over 3 gates
          (valid because L_b == 0; a per-gate-bias fallback is kept),
          si*tg on GpSimd, rest of cell math on DVE
  - head: b = 4*tanh((h@L_Wo+bo)/2), p = sigmoid(4 * a4 * (theta - b4))

Outputs [1, P] per core are gathered and inverse-permuted on the host.
"""

import sys
import hashlib

import numpy as np
import ml_dtypes

for _p in ("/opt/trn_rl_repo",):
    if _p not in sys.path:
        sys.path.insert(0, _p)

B, T, K, D, H, HL, S, KN = 4096, 50, 32, 64, 256, 128, 100000, 1000
N_CORES = 8
NEG = -1.0e9

_state = {}

BF16 = ml_dtypes.bfloat16


# ---------------------------------------------------------------- host prep

def _host_prep(inputs):
    """Sort/deal/pad students; build per-core input arrays + schedule."""
    lens = np.asarray(inputs["qid_len"]).astype(np.int64)          # [B]
    counts = np.bincount(lens, minlength=T + 1)                    # index 0..50
    m = -(-counts // N_CORES)                                      # ceil
    m[0] = 0
    m[1:] += m[1:] % 2                                             # even counts
    P0 = int(m[1:].sum())
    P = ((P0 + 15) // 16) * 16
    m[1] += P - P0                                                 # 16|P-P0 even

    # per-core identical length profile, descending
    profile = np.repeat(np.arange(T, 0, -1), m[T:0:-1])            # [P]
    assert profile.shape[0] == P

    students = -np.ones((N_CORES, P), np.int64)
    ptr = 0
    for l in range(T, 0, -1):
        idxs = np.where(lens == l)[0]
        for c in range(N_CORES):
            take = idxs[c::N_CORES]
            assert take.shape[0] <= m[l]
            students[c, ptr:ptr + take.shape[0]] = take
        ptr += m[l]
    assert ptr == P

    n_t = np.array([(profile > t).sum() for t in range(T)], np.int64)
    assert np.all(n_t % 2 == 0)
    n2 = [int(x) // 2 for x in n_t]

    ND, NB = P // 4, P // 16

    qidemb = np.asarray(inputs["qidemb"], np.float32)
    stuE = np.asarray(inputs["stuE"], np.float32)
    uididx = np.asarray(inputs["uididx"])
    kcodeidx = np.asarray(inputs["kcodeidx"])
    kcode_len = np.asarray(inputs["kcode_len"]).astype(np.int64)

    lb_np = np.asarray(inputs["L_b"], np.float32)
    bias_zero = bool(np.all(lb_np == 0.0))

    # weights (replicated)
    wts = {
        "wi": np.asarray(inputs["L_Wi"], np.float32).astype(BF16),          # [64,512]
        "wh": np.asarray(inputs["L_Wh"], np.float32).astype(BF16),          # [128,512]
        "tw1": np.asarray(inputs["T_W1"], np.float32).astype(BF16),         # [64,256]
        "aw1": np.asarray(inputs["A_W1"], np.float32).astype(BF16),
        "tw2": np.asarray(inputs["T_W2"], np.float32)[:, 0].reshape(2, 128).T.copy().astype(BF16),
        "aw2": np.asarray(inputs["A_W2"], np.float32)[:, 0].reshape(2, 128).T.copy().astype(BF16),
        "lwo": np.asarray(inputs["L_Wo"], np.float32).reshape(128, 1).astype(BF16),
        "lb": lb_np.reshape(4, 128).T.copy(),
        "tb1": np.asarray(inputs["T_b1"], np.float32).reshape(2, 128).T.copy(),
        "ab1": np.asarray(inputs["A_b1"], np.float32).reshape(2, 128).T.copy(),
        "scal": np.array([[float(np.asarray(inputs["T_b2"]).reshape(-1)[0]),
                           float(np.asarray(inputs["A_b2"]).reshape(-1)[0]),
                           0.5 * float(np.asarray(inputs["L_bo"]).reshape(-1)[0])]],
                         np.float32),
    }
    kne_bf = np.asarray(inputs["knE"], np.float32).astype(BF16)             # [1000,64]
    # constant pattern tiles
    sumpat = np.zeros((128, 4), BF16)
    for a in range(4):
        sumpat[32 * a:32 * (a + 1), a] = 1
    bck = np.zeros((4, 128), np.float32)       # block-broadcast: row a -> rows 32a..
    for a in range(4):
        bck[a, 32 * a:32 * (a + 1)] = 1
    blk4 = np.zeros((4, 204), BF16)
    for j in range(4):
        blk4[j, 51 * j:51 * (j + 1)] = 1
    consts = {"sumpat": sumpat, "bck": bck, "blk4": blk4}

    in_maps = []
    for c in range(N_CORES):
        sel = students[c]
        safe = np.where(sel >= 0, sel, 0)

        q = qidemb[safe]                                           # [P,50,64]
        st = stuE[uididx[safe]]                                    # [P,64]
        qid_plus = np.concatenate([q, st[:, None, :]], axis=1).astype(BF16)
        # host-side transpose: qT[G, d, 102*p8 + 51*par + t]
        qT = qid_plus.reshape(NB, 8, 2, 51, 64).transpose(0, 4, 1, 2, 3) \
            .reshape(NB, 64, 816)

        # pre-gathered knowledge embeddings in both matmul layouts:
        # kemb2[G, 32a+k, 64dd+d] (stage-2 lhsT), kembT[G, d, 128dd+32a+k]
        ke = kne_bf[kcodeidx[safe]]                                # [P,32,64]
        keb = ke.reshape(NB, 4, 4, 32, 64)                         # G,dd,a,k,d
        kemb2 = keb.transpose(0, 2, 3, 1, 4).reshape(NB, 128, 256)
        kembT = keb.transpose(0, 4, 1, 2, 3).reshape(NB, 64, 512)

        # single packed per-G input: [128, 1584]
        #   rows 0:64 cols 0:816    qT
        #   rows 0:64 cols 816:1328 kembT
        #   rows 0:128 cols 1328:1584 kemb2
        qkk = np.zeros((NB, 128, 1584), BF16)
        qkk[:, 0:64, 0:816] = qT
        qkk[:, 0:64, 816:1328] = kembT
        qkk[:, :, 1328:1584] = kemb2

        kl = kcode_len[safe].reshape(ND, 4)                        # [ND,4]
        kk = np.arange(K)
        kmf3 = (kk[None, None, :] < kl[:, :, None])                # [ND,4,32]
        kmf = kmf3.transpose(1, 2, 0).reshape(128, ND).astype(np.float32)
        # brow4[j, G, m] = bias of duo 4G+j at out1 row m (0 valid / -1e9 invalid)
        brow4 = np.where(kmf3, 0.0, NEG).reshape(NB, 4, 128).transpose(1, 0, 2) \
            .reshape(4, NB * 128).copy().astype(BF16)

        im = {"qkk": qkk, "kmf": kmf, "brow": brow4}
        im.update(wts)
        im.update(consts)
        in_maps.append(im)

    meta = {"P": P, "n2": n2, "students": students, "bias_zero": bias_zero}
    return in_maps, meta


# ---------------------------------------------------------------- program

def _build_program(P, n2, bias_zero, phases=("attn", "dnn", "lstm", "head")):
    import os as _os
    if _os.environ.get("KPHASES"):
        phases = tuple(_os.environ["KPHASES"].split(","))
    ATT = int(_os.environ.get("KATT", "9"))
    import concourse.bacc as bacc
    import concourse.bass as bass
    import concourse.tile as tile
    from concourse import mybir
    from concourse.tile import add_dep_helper as add_dep
    from contextlib import ExitStack

    dt = mybir.dt
    AF = mybir.ActivationFunctionType
    ND, NB, NP2 = P // 4, P // 16, P // 2

    nc = bacc.Bacc("TRN2", target_bir_lowering=False, debug=False,
                   enable_asserts=False)

    qkk_d = nc.dram_tensor("qkk", [NB, 128, 1584], dt.bfloat16,
                           kind="ExternalInput")
    kmf_d = nc.dram_tensor("kmf", [128, ND], dt.float32, kind="ExternalInput")
    brow_d = nc.dram_tensor("brow", [4, NB * 128], dt.bfloat16, kind="ExternalInput")
    wi_d = nc.dram_tensor("wi", [64, 512], dt.bfloat16, kind="ExternalInput")
    wh_d = nc.dram_tensor("wh", [128, 512], dt.bfloat16, kind="ExternalInput")
    tw1_d = nc.dram_tensor("tw1", [64, 256], dt.bfloat16, kind="ExternalInput")
    aw1_d = nc.dram_tensor("aw1", [64, 256], dt.bfloat16, kind="ExternalInput")
    tw2_d = nc.dram_tensor("tw2", [128, 2], dt.bfloat16, kind="ExternalInput")
    aw2_d = nc.dram_tensor("aw2", [128, 2], dt.bfloat16, kind="ExternalInput")
    lwo_d = nc.dram_tensor("lwo", [128, 1], dt.bfloat16, kind="ExternalInput")
    lb_d = nc.dram_tensor("lb", [128, 4], dt.float32, kind="ExternalInput")
    tb1_d = nc.dram_tensor("tb1", [128, 2], dt.float32, kind="ExternalInput")
    ab1_d = nc.dram_tensor("ab1", [128, 2], dt.float32, kind="ExternalInput")
    scal_d = nc.dram_tensor("scal", [1, 3], dt.float32, kind="ExternalInput")
    sumpat_d = nc.dram_tensor("sumpat", [128, 4], dt.bfloat16, kind="ExternalInput")
    bck_d = nc.dram_tensor("bck", [4, 128], dt.float32, kind="ExternalInput")
    blk4_d = nc.dram_tensor("blk4", [4, 204], dt.bfloat16, kind="ExternalInput")
    out_d = nc.dram_tensor("out", [1, P], dt.float32, kind="ExternalOutput")

    with tile.TileContext(nc) as tc, ExitStack() as ctx:
        const = ctx.enter_context(tc.tile_pool(name="const", bufs=1))
        state = ctx.enter_context(tc.tile_pool(name="state", bufs=1))

        def load(pool, shape, dty, dram, dma2=False):
            t = pool.tile(shape, dty, tag=f"c_{dram.name}", name=f"c_{dram.name}")
            if dma2:  # duplicate 64-row weight into both partition halves
                nc.sync.dma_start(t[0:64, :], dram.ap())
                nc.sync.dma_start(t[64:128, :], dram.ap())
            else:
                nc.sync.dma_start(t[:], dram.ap())
            return t

        kmf_t = load(const, [128, ND], dt.float32, kmf_d)
        brow_t = load(const, [4, NB * 128], dt.bfloat16, brow_d)
        wi_t = load(const, [128, 512], dt.bfloat16, wi_d, dma2=True)
        wh_t = load(const, [128, 512], dt.bfloat16, wh_d)
        tw1_t = load(const, [128, 256], dt.bfloat16, tw1_d, dma2=True)
        aw1_t = load(const, [128, 256], dt.bfloat16, aw1_d, dma2=True)
        tw2_t = load(const, [128, 2], dt.bfloat16, tw2_d)
        aw2_t = load(const, [128, 2], dt.bfloat16, aw2_d)
        lwo_t = load(const, [128, 1], dt.bfloat16, lwo_d)
        lb_t = load(const, [128, 4], dt.float32, lb_d)
        tb1_t = load(const, [128, 2], dt.float32, tb1_d)
        ab1_t = load(const, [128, 2], dt.float32, ab1_d)
        scal_t = load(const, [1, 3], dt.float32, scal_d)
        sumpat_t = load(const, [128, 4], dt.bfloat16, sumpat_d)
        bck_t = load(const, [4, 128], dt.float32, bck_d)
        blk4_t = load(const, [4, 204], dt.bfloat16, blk4_d)

        # persistent tensors
        # bvec is STUDENT-major: col = pair*T + t (pair = 8G+2dd+par), half = g%2
        bvec = state.tile([128, NP2 * T], dt.bfloat16)
        mastav = state.tile([128, P], dt.bfloat16)       # [d(half), 2*slot+c]
        h_t = state.tile([128, P], dt.bfloat16)
        c_t = state.tile([128, P], dt.float32)
        theta_t = state.tile([1, P], dt.float32)
        a4_t = state.tile([1, P], dt.float32)
        b4_t = state.tile([1, P], dt.float32)
        res_t = state.tile([1, P], dt.float32)
        nc.vector.memset(h_t[:], 0.0)
        nc.vector.memset(c_t[:], 0.0)

        # ---------------- attention ----------------
        with ExitStack() as atx:
          if "attn" in phases:
            qin = atx.enter_context(tc.tile_pool(name="qin", bufs=4))
            sbA = atx.enter_context(tc.tile_pool(name="sbA", bufs=4))
            ps_1 = atx.enter_context(tc.tile_pool(name="ps_1", bufs=2, space="PSUM"))
            ps_2e = atx.enter_context(tc.tile_pool(name="ps_2e", bufs=2, space="PSUM"))
            ps_2o = atx.enter_context(tc.tile_pool(name="ps_2o", bufs=2, space="PSUM"))
            ps_d = atx.enter_context(tc.tile_pool(name="ps_d", bufs=1, space="PSUM"))
            ps_b = atx.enter_context(tc.tile_pool(name="ps_b", bufs=1, space="PSUM"))

            bvf2 = bvec[:].rearrange("p (q two t) -> p q two t", two=2, t=T)

            for G in range(NB):
                # packed per-G input: qT | kembT | kemb2
                qkk = qin.tile([128, 1584], dt.bfloat16, tag="qkk")
                if ATT >= 1:
                    nc.sync.dma_start(qkk[:], qkk_d.ap()[G])
                qT = qkk[0:64, 0:816]
                kT = qkk[0:64, 816:1328]
                kc16 = qkk[:, 1328:1584]
                if ATT < 4:
                    continue

                # stage 1: out1[32a+k, 51*dd+t] = scoresT (+ mastery col 50)
                # first writer: bias matmul filling the whole bank with the
                # -1e9 invalid-k bias (start=True), then 16 quadrant matmuls
                # accumulate the actual scores.
                if ATT < 5:
                    continue
                out1 = ps_1.tile([128, 512], dt.float32, tag="out1")
                bmm = nc.tensor.matmul(
                    out1[:, 0:204], brow_t[:, 128 * G:128 * (G + 1)], blk4_t[:],
                    start=True, stop=False, skip_group_check=True)
                for dd in range(4):
                    for a in range(4):
                        pr = 2 * dd + a // 2
                        rhs = qT[:, 102 * pr + 51 * (a % 2):102 * pr + 51 * (a % 2) + 51]
                        mm = nc.tensor.matmul(
                            out1[32 * a:32 * (a + 1), 51 * dd:51 * (dd + 1)],
                            kT[:, 128 * dd + 32 * a:128 * dd + 32 * (a + 1)],
                            rhs, start=False, stop=(dd == 3 and a == 3),
                            tile_position=(0, 32 * a), skip_group_check=True)
                        add_dep(mm.ins, bmm.ins, reason="bias first-writer")

                # softmax pieces
                if ATT < 6:
                    continue
                o1v = out1[:, 0:204].rearrange("p (d c) -> p d c", d=4)
                expw = sbA.tile([128, 208], dt.bfloat16, tag="expw")
                ewv = expw[:].rearrange("p (d c) -> p d c", d=4)
                nc.scalar.activation(ewv[:, :, 0:50], o1v[:, :, 0:50],
                                     AF.Exp, scale=0.15)
                # mastery sigmoid via exp so Scalar never swaps ACT tables:
                # sigmoid(x) = e^x / (1 + e^x); e^x underflows to 0 for the
                # -1e9-masked invalid-k rows (recip input stays in [1, ~21])
                me = sbA.tile([128, 4], dt.float32, tag="me")
                ma = sbA.tile([128, 4], dt.float32, tag="ma")
                nc.scalar.activation(me[:], o1v[:, :, 50:51].rearrange("p a o -> p (a o)"),
                                     AF.Exp, scale=0.2)
                nc.gpsimd.tensor_scalar_add(ma[:], me[:], 1.0)
                with nc.allow_low_precision(reason="mastery sigmoid recip"):
                    nc.vector.reciprocal_approx_fast(ma[:], ma[:])
                nc.gpsimd.tensor_mul(me[:], me[:], ma[:])
                nc.gpsimd.tensor_mul(ewv[:, :, 50:51].rearrange("p a o -> p (a o)"),
                                     me[:], kmf_t[:, 4 * G:4 * G + 4])
                nc.gpsimd.tensor_copy(ewv[:, :, 51:52].rearrange("p a o -> p (a o)"),
                                      kmf_t[:, 4 * G:4 * G + 4])

                # denominators -> reciprocal -> fold into expw
                if ATT < 7:
                    continue
                dps = ps_d.tile([4, 208], dt.float32, tag="dps")
                nc.tensor.matmul(dps[:], sumpat_t[:], expw[:],
                                 start=True, stop=True, skip_group_check=True)
                rden = sbA.tile([4, 208], dt.float32, tag="rden")
                with nc.allow_low_precision(reason="softmax denominators"):
                    nc.vector.reciprocal_approx_fast(rden[:], dps[:])
                # mast (col 50) and kmf (col 51) stay unnormalized
                nc.gpsimd.memset(
                    rden[:].rearrange("p (d c) -> p d c", d=4)[:, :, 50:52], 1.0)
                bc = ps_b.tile([128, 208], dt.float32, tag="bc")
                nc.tensor.matmul(bc[:], bck_t[:], rden[:],
                                 start=True, stop=True, skip_group_check=True)
                nc.vector.tensor_mul(expw[:], expw[:], bc[:])

                # stage 2: [bvecT | mastvec | avec]; expw already normalized,
                # so out2 is the final bvecT. first writer per partition half
                # is start=True (a=0/a=1)
                if ATT < 8:
                    continue
                out2e = ps_2e.tile([128, 512], dt.float32, tag="out2e")
                out2o = ps_2o.tile([128, 512], dt.float32, tag="out2o")
                out2_par = (out2e, out2o)
                firsts = [[None, None], [None, None]]   # [par][hh]
                for dd in range(4):
                    for a in range(4):
                        par = a // 2
                        hh = a % 2
                        o2 = out2_par[par]
                        mm = nc.tensor.matmul(
                            o2[64 * hh:64 * hh + 64, 52 * dd:52 * (dd + 1)],
                            kc16[32 * a:32 * (a + 1), 64 * dd:64 * (dd + 1)],
                            expw[32 * a:32 * (a + 1), 52 * dd:52 * (dd + 1)],
                            start=(firsts[par][hh] is None), stop=True,
                            tile_position=(32 * a, 64 * hh),
                            skip_group_check=True)
                        if firsts[par][hh] is None:
                            firsts[par][hh] = mm
                        else:
                            add_dep(mm.ins, firsts[par][hh].ins,
                                    reason="bank first-writer")

                # write bvec (cast): one [128,4,50] copy per parity; both
                # partition halves of out2 land in the same pair columns
                if ATT < 9:
                    continue
                for par in range(2):
                    src = out2_par[par][:, 0:208].rearrange(
                        "p (d c) -> p d c", d=4)[:, :, 0:50]
                    nc.vector.tensor_copy(bvf2[:, 4 * G:4 * G + 4, par, :], src)

                # mastvec/avec extraction
                mavdst = mastav[:, 16 * G:16 * (G + 1)].rearrange(
                    "p (blk q c) -> p blk q c", blk=4, q=2)
                for par in range(2):
                    src = out2_par[par][:, 0:208].rearrange(
                        "p (d c) -> p d c", d=4)[:, :, 50:52]
                    nc.vector.tensor_copy(mavdst[:, :, par, :], src)

        # ---------------- theta / a DNNs ----------------
        with ExitStack() as dtx:
          if "dnn" in phases:
            sbD = dtx.enter_context(tc.tile_pool(name="sbD", bufs=2))
            ps_h = dtx.enter_context(tc.tile_pool(name="ps_h", bufs=2, space="PSUM"))
            ps_o = dtx.enter_context(tc.tile_pool(name="ps_o", bufs=2, space="PSUM"))

            mav = mastav[:].rearrange("p (s c) -> p s c", s=NP2)
            for net, (w1, b1, w2, sc) in enumerate(
                    [(tw1_t, tb1_t, tw2_t, 0), (aw1_t, ab1_t, aw2_t, 1)]):
                dstv = (theta_t if net == 0 else a4_t)[:].rearrange(
                    "o (q two) -> o q two", two=2)
                for half in range(2):
                    r = slice(64 * half, 64 * half + 64)
                    rhs = mav[r, :, net]                       # [64, NP2]
                    ops = ps_o.tile([1, 512], dt.float32, tag="ops")
                    omm0 = None
                    for b in range(2):
                        hps = ps_h.tile([128, 512], dt.float32, tag="hps")
                        nc.tensor.matmul(hps[:, 0:NP2],
                                         w1[r, 128 * b:128 * (b + 1)], rhs,
                                         start=True, stop=True,
                                         tile_position=(64 * half, 0))
                        t1b = sbD.tile([128, NP2], dt.bfloat16, tag="t1b")
                        nc.scalar.activation(t1b[:], hps[:, 0:NP2], AF.Tanh,
                                             bias=b1[:, b:b + 1])
                        omm = nc.tensor.matmul(ops[:, 0:NP2], w2[:, b:b + 1], t1b[:],
                                               start=(b == 0), stop=(b == 1),
                                               skip_group_check=True)
                        if b == 0:
                            omm0 = omm
                        else:
                            add_dep(omm.ins, omm0.ins,
                                    reason="accum first-writer")
                    nc.scalar.activation(dstv[:, :, half], ops[:, 0:NP2],
                                         AF.Identity, bias=scal_t[:, sc:sc + 1])
            # a4 = tanh(|a_pre| / 2)
            nc.scalar.activation(a4_t[:], a4_t[:], AF.Abs)
            nc.scalar.activation(a4_t[:], a4_t[:], AF.Tanh, scale=0.5)

        # ---------------- LSTM ----------------
        # Two bank layouts over the 8 PSUM banks:
        #  - early steps (3n > 512): per group a 3-bank [i|f|o] tile (ACT
        #    reads stride across banks) + a g bank
        #  - late steps (3n <= 512): gates PACKED at cols [0, n, 2n] of ONE
        #    bank per (group, t-parity) + g bank per (group, t-parity);
        #    contiguous ACT reads and ping-pong so step t+1's matmuls never
        #    wait on step t's activations
        bvf = bvec[:].rearrange("p (q t) -> p q t", t=T)

        def cell_math(sbL, grp, t, n, cb, sig_src, g_src, sfo_off):
            """sigmoid/tanh + cell update for one (grp, t) column block.
            sig_src: [128, 3, n]-shaped AP (i,f,o pre-acts); g_src [128, n]."""
            sfo = sbL.tile([128, 3 * NP2], dt.float32, tag=f"sfo{grp}")
            tg = sbL.tile([128, NP2], dt.float32, tag=f"tg{grp}")
            dst3 = sfo[:, sfo_off:sfo_off + 3 * n].rearrange(
                "p (j c) -> p j c", j=3)
            if bias_zero:
                nc.scalar.activation(dst3, sig_src, AF.Sigmoid)
                nc.scalar.activation(tg[:, 0:n], g_src, AF.Tanh)
            else:
                for j, b in ((0, 0), (1, 1), (2, 3)):
                    nc.scalar.activation(dst3[:, j], sig_src[:, j],
                                         AF.Sigmoid, bias=lb_t[:, b:b + 1])
                nc.scalar.activation(tg[:, 0:n], g_src, AF.Tanh,
                                     bias=lb_t[:, 2:3])
            si = sfo[:, sfo_off:sfo_off + n]
            sf = sfo[:, sfo_off + n:sfo_off + 2 * n]
            so = sfo[:, sfo_off + 2 * n:sfo_off + 3 * n]
            t1 = sbL.tile([128, NP2], dt.float32, tag=f"t1{grp}")
            nc.vector.tensor_mul(t1[:, 0:n], si, tg[:, 0:n])
            nc.vector.tensor_mul(c_t[:, cb:cb + n], c_t[:, cb:cb + n], sf)
            nc.vector.tensor_add(c_t[:, cb:cb + n], c_t[:, cb:cb + n],
                                 t1[:, 0:n])
            tc2 = sbL.tile([128, NP2], dt.float32, tag=f"tc2{grp}")
            nc.scalar.activation(tc2[:, 0:n], c_t[:, cb:cb + n], AF.Tanh)
            nc.vector.tensor_mul(h_t[:, cb:cb + n], so, tc2[:, 0:n])

        def gate_mms(gp_off_b, grp, t, n, cb):
            """Wi (start) + Wh (accum) matmuls for the i,f,o,g regions."""
            wr = 64 * grp
            xrhs = bvf[wr:wr + 64, 0:n, t]
            for (gp, off, b) in gp_off_b:
                mmi = nc.tensor.matmul(gp[:, off:off + n],
                                       wi_t[wr:wr + 64, 128 * b:128 * (b + 1)],
                                       xrhs,
                                       start=True, stop=False,
                                       tile_position=(wr, 0),
                                       skip_group_check=True)
                mmh = nc.tensor.matmul(gp[:, off:off + n],
                                       wh_t[:, 128 * b:128 * (b + 1)],
                                       h_t[:, cb:cb + n],
                                       start=False, stop=True,
                                       skip_group_check=True)
                add_dep(mmh.ins, mmi.ins, reason="accum first-writer")

        t_late = next((t for t in range(T) if 3 * n2[t] <= 512), T)

        if "lstm" in phases:
          with ExitStack() as ltx:
            ps_g = ltx.enter_context(tc.tile_pool(name="ps_g", bufs=1, space="PSUM"))
            sbL = ltx.enter_context(tc.tile_pool(name="sbL", bufs=2))
            gifo = {}
            gg = {}
            for grp in (0, 1):
                gifo[grp] = ps_g.tile([128, 1536], dt.float32,
                                      tag=f"ifo{grp}", name=f"ifo{grp}")
                gg[grp] = ps_g.tile([128, 512], dt.float32,
                                    tag=f"g{grp}", name=f"g{grp}")
            for t in range(t_late):
                n = n2[t]
                if n == 0:
                    continue
                for grp in (0, 1):
                    cb = NP2 * grp
                    regions = [(gifo[grp], 0, 0), (gifo[grp], 512, 1),
                               (gifo[grp], 1024, 3), (gg[grp], 0, 2)]
                    gate_mms(regions, grp, t, n, cb)
                    giv = gifo[grp][:].rearrange("p (j c) -> p j c", j=3)
                    cell_math(sbL, grp, t, n, cb,
                              giv[:, :, 0:n], gg[grp][:, 0:n], 0)

          with ExitStack() as ltx:
            ps_g = ltx.enter_context(tc.tile_pool(name="ps_g2", bufs=1, space="PSUM"))
            sbL = ltx.enter_context(tc.tile_pool(name="sbL2", bufs=2))
            gifo = {}
            gg = {}
            for grp in (0, 1):
                for par in (0, 1):
                    gifo[(grp, par)] = ps_g.tile(
                        [128, 512], dt.float32,
                        tag=f"pifo{grp}{par}", name=f"pifo{grp}{par}")
                    gg[(grp, par)] = ps_g.tile(
                        [128, 512], dt.float32,
                        tag=f"pg{grp}{par}", name=f"pg{grp}{par}")
            for t in range(t_late, T):
                n = n2[t]
                if n == 0:
                    continue
                par = t % 2
                for grp in (0, 1):
                    cb = NP2 * grp
                    ifo = gifo[(grp, par)]
                    gt = gg[(grp, par)]
                    regions = [(ifo, 0, 0), (ifo, n, 1),
                               (ifo, 2 * n, 3), (gt, 0, 2)]
                    gate_mms(regions, grp, t, n, cb)
                    sig_src = ifo[:, 0:3 * n].rearrange("p (j c) -> p j c", j=3)
                    cell_math(sbL, grp, t, n, cb, sig_src, gt[:, 0:n], 0)

        # ---------------- head + combine ----------------
        with ExitStack() as htx:
          if "head" in phases:
            ps_r = htx.enter_context(tc.tile_pool(name="ps_r", bufs=2, space="PSUM"))
            sbH = htx.enter_context(tc.tile_pool(name="sbH", bufs=2))
            b4v = b4_t[:].rearrange("o (q two) -> o q two", two=2)
            for half in range(2):
                bps = ps_r.tile([1, 512], dt.float32, tag="bps")
                nc.tensor.matmul(bps[:, 0:NP2], lwo_t[:],
                                 h_t[:, NP2 * half:NP2 * (half + 1)],
                                 start=True, stop=True)
                nc.scalar.activation(b4v[:, :, half], bps[:, 0:NP2], AF.Tanh,
                                     scale=0.5, bias=scal_t[:, 2:3])
            d1 = sbH.tile([1, P], dt.float32, tag="d1")
            # p = sigmoid(a*(t-b)) with a = 4*a4, b = 4*b4
            #   = sigmoid(4 * a4 * (theta - 4*b4))
            nc.vector.scalar_tensor_tensor(d1[:], b4_t[:], -4.0, theta_t[:],
                                           mybir.AluOpType.mult,
                                           mybir.AluOpType.add)
            nc.vector.tensor_mul(d1[:], d1[:], a4_t[:])
            nc.scalar.activation(res_t[:], d1[:], AF.Sigmoid, scale=4.0)
            nc.sync.dma_start(out_d.ap(), res_t[:])

    nc.compile()
    return nc


# ---------------------------------------------------------------- runner

def _fingerprint(inputs):
    h = hashlib.md5()
    for k in sorted(inputs):
        a = np.asarray(inputs[k])
        h.update(k.encode())
        h.update(str(a.shape).encode())
        h.update(str(a.dtype).encode())
        flat = a.reshape(-1)
        stride = max(1, flat.size // 65536)
        h.update(np.ascontiguousarray(flat[::stride]).tobytes())
    return h.digest()


def _install_ntff_hook():
    """Provide antenv.axon_hooks (NTFF profiling over the axon tunnel) when
    the image lacks it: drives libaxon_pjrt.so's profile ABI via ctypes,
    mirroring trn_boot._ntff_profile_via_ctypes."""
    import types
    import ctypes
    import contextlib
    try:
        from antenv.axon_hooks import get_axon_ntff_profile_hook  # noqa: F401
        return True
    except ImportError:
        pass
    so_path = "/opt/axon/libaxon_pjrt.so"
    try:
        lib = ctypes.CDLL(so_path)
    except OSError:
        return False
    if not hasattr(lib, "axon_start_nrt_profile"):
        return False
    lib.axon_start_nrt_profile.argtypes = [ctypes.POINTER(ctypes.c_int64),
                                           ctypes.c_size_t]
    lib.axon_start_nrt_profile.restype = ctypes.c_int64
    lib.axon_stop_nrt_profile.argtypes = [ctypes.c_char_p]
    lib.axon_stop_nrt_profile.restype = ctypes.c_int64

    @contextlib.contextmanager
    def _hook(output_dir, device_ids):
        import jax
        jax.devices()
        if device_ids:
            ids = (ctypes.c_int64 * len(device_ids))(*device_ids)
            rc = lib.axon_start_nrt_profile(ids, len(device_ids))
        else:
            rc = lib.axon_start_nrt_profile(None, 0)
        if rc != 0:
            raise RuntimeError(f"axon_start_nrt_profile rc={rc}")
        try:
            yield
        finally:
            n = lib.axon_stop_nrt_profile(str(output_dir).encode())
            if n < 0:
                raise RuntimeError(f"axon_stop_nrt_profile rc={n}")

    mod = types.ModuleType("antenv.axon_hooks")
    mod.get_axon_ntff_profile_hook = lambda: _hook
    mod.set_axon_ntff_profile_hook = lambda h: None
    import antenv
    sys.modules["antenv.axon_hooks"] = mod
    antenv.axon_hooks = mod
    return True


def profile(trace=True, trace_cores=None):
    """Run the cached program with NTFF tracing; returns BassKernelResults
    (exec_time_ns = on-device NEFF execution time). Call kernel() first."""
    import concourse.bass_utils as bu
    assert "nc" in _state, "call kernel() first to build/caches the program"
    _install_ntff_hook()
    bu.upload_artifacts = lambda d: "local"   # no artifact bucket here
    return bu.run_bass_kernel_spmd(_state["nc"], _state["in_maps"],
                                   core_ids=list(range(N_CORES)), trace=trace,
                                   trace_cores=trace_cores)


def kernel(**inputs):
    from concourse.bass_utils import run_bass_kernel_spmd

    fp = _fingerprint(inputs)
    cached = _state.get("fp")
    if cached != fp:
        in_maps, meta = _host_prep(inputs)
        key = (meta["P"], tuple(meta["n2"]), meta["bias_zero"])
        if _state.get("prog_key") != key:
            _state["nc"] = _build_program(meta["P"], meta["n2"],
                                          meta["bias_zero"])
            _state["prog_key"] = key
        _state["in_maps"] = in_maps
        _state["meta"] = meta
        _state["fp"] = fp

    meta = _state["meta"]
    res = run_bass_kernel_spmd(_state["nc"], _state["in_maps"],
                               core_ids=list(range(N_CORES)))
    out = np.zeros((B, 1), np.float32)
    students = meta["students"]
    for c in range(N_CORES):
        r = res.results[c]["out"].reshape(-1)
        sel = students[c]
        valid = sel >= 0
        out[sel[valid], 0] = r[:len(sel)][valid]
    return out
